# revision 2
# baseline (speedup 1.0000x reference)
"""Trainium2 Bass kernel for pairwise-scores CoreNet — v3.

scores[i, j] = (e_i @ wa) + (e_j @ wb) + sum_d wc_d * |e_id - e_jd| + b

Device computes only the O(N^2 D) pair term; every O(N D) derived constant
(scaled transposed table, rank-1 rows, sign windows) is prepared host-side
and DMA'd in.

Structure (per core, rows interleaved i mod 8 == c):
  * upper block-triangle only: local row k computes cols j >= (k//16)*128;
    lower blocks come from on-device PE transposes of the computed blocks
    with the rank-1 asymmetry (u_j - u_i, u = sa - sb) patched in.
  * absd tiles: DVE computes 2*max(e~_dj, e~_dk) (corrections folded in the
    PSUM preload); every 4th wide row is produced on the ACT engine as
    |e~_dj - e~_dk| directly (no corrections; preload masked per row).
  * PE reduces over d via sliding one-hot sign windows into PSUM, one
    useful output row per matmul.
"""

import sys

sys.path.insert(0, "/opt/trn_rl_repo")

from contextlib import ExitStack

import numpy as np

import concourse.bass as bass
import concourse.mybir as mybir
import concourse.tile as tile
from concourse import bacc
from concourse.bass_utils import run_bass_kernel_spmd

F32 = mybir.dt.float32
BF16 = mybir.dt.bfloat16
Alu = mybir.AluOpType
Act = mybir.ActivationFunctionType

N_CORES = 8
N, D, R = 1024, 256, 128
H = D // 128          # 2 d-tiles
NB = N // 128         # 8 column blocks
G = R // NB           # 16 rows per block-group

# ACT rows: every 4th row (narrow A=7 fillers excluded); aligned so each
# post-drain row is an ACT row (frees the DVE for the drain copies). k=0
# stays on DVE: it gates the pipeline start and ACT is 3x slower per tile.
ACT_ROW = [k % 4 == 0 and 0 < k < 112 for k in range(R)]

# processing order: groups A=0..6 sequential (so PSUM blocks complete in
# order), with the 16 narrowest rows (A=7) interleaved one per 4 rows into
# the A=0..3 sections, which have DVE slack to absorb them
ORDER = []
for t in range(16):
    ORDER.extend(range(4 * t, 4 * (t + 1)))
    ORDER.append(112 + t)
ORDER.extend(range(64, 112))
assert sorted(ORDER) == list(range(128))

# strip layout (bf16 row constants)
S_SBQ = 0          # [0:1024)      sbq_j = sb_j - q_j + b
S_QB = 1024        # [1024:1152)   -qb_loc
S_UL = 1152        # [1152:1280)   -u_loc
S_Q = 1280         # [1280:2304)   q_j
S_MASK = 2304      # [2304:2432)   mask_act (1.0 on ACT rows)
S_ONES = 2432      # [2432:2944)   ones (512 wide: used as an F=512 rhs)
S_LEN = 2944

OFF2 = [8 * B * (B - 1) for B in range(NB)]   # out2 col offset per block B
W2 = [G * B for B in range(NB)]               # out2 width per block B


def build_program() -> bass.Bass:
    n, d, r = N, D, R
    nc = bacc.Bacc("TRN2", target_bir_lowering=False, debug=False)

    ett_dram = nc.dram_tensor("ett", [128, H * n], BF16, kind="ExternalInput")
    ebt_dram = nc.dram_tensor("ebtt", [128, H * 128], F32, kind="ExternalInput")
    strip_dram = nc.dram_tensor("strip", [1, S_LEN], BF16, kind="ExternalInput")
    unat_dram = nc.dram_tensor("unat", [128, NB], F32, kind="ExternalInput")
    sacol_dram = nc.dram_tensor("sacol", [128, 1], F32, kind="ExternalInput")
    win_dram = nc.dram_tensor("wint", [128, H * 256], BF16, kind="ExternalInput")
    ident_dram = nc.inline_tensor(np.eye(128, dtype=np.float32), name="ident128")

    with tile.TileContext(nc) as tc, ExitStack() as ctx:
        const = ctx.enter_context(tc.tile_pool(name="const", bufs=1))
        absd_pool = ctx.enter_context(tc.tile_pool(name="absd", bufs=5))
        ps_acc = ctx.enter_context(tc.tile_pool(name="psacc", bufs=1, space="PSUM"))
        ps_misc = ctx.enter_context(tc.tile_pool(name="psmisc", bufs=1, space="PSUM"))

        # ---------------- DMA loads (spread across 3 issuing engines) ----------------
        # critical chain: win + ett piece 0 + ebt -> absd(k=0) -> first
        # matmuls. ACT issues no DMAs (its 1.3us act-table load would delay
        # them); the rest ride the Pool queue.
        win = const.tile([128, H * 256], BF16)
        nc.sync.dma_start(out=win[:, :], in_=win_dram.ap())
        e_t2 = const.tile([128, H * n], BF16)      # e_t2[:, h*n + j]
        for piece in range(2):
            nc.sync.dma_start(
                out=e_t2[:, piece * 512 : (piece + 1) * 512],
                in_=ett_dram.ap()[:, piece * 512 : (piece + 1) * 512],
            )

        ebt = const.tile([128, H * 128], F32)      # ebt[:, h*128 + k]
        nc.gpsimd.dma_start(out=ebt[:, :], in_=ebt_dram.ap())
        for piece in range(2, 4):
            nc.gpsimd.dma_start(
                out=e_t2[:, piece * 512 : (piece + 1) * 512],
                in_=ett_dram.ap()[:, piece * 512 : (piece + 1) * 512],
            )
        rows = const.tile([1, S_LEN], BF16)
        nc.gpsimd.dma_start(out=rows[0:1, :], in_=strip_dram.ap())
        ident32 = const.tile([128, 128], F32)
        nc.gpsimd.dma_start(out=ident32[:, :], in_=ident_dram.ap())
        u_nat = const.tile([128, NB], F32)
        nc.gpsimd.dma_start(out=u_nat[:, :], in_=unat_dram.ap())
        sa_col = const.tile([128, 1], F32)
        nc.gpsimd.dma_start(out=sa_col[:, :], in_=sacol_dram.ap())

        def et(h):
            return e_t2[:, h * n : (h + 1) * n]

        def srow(off, w):
            return rows[0:1, off : off + w]

        # ---------------- PSUM preload ----------------
        accs = [
            ps_acc.tile([128, 512], F32, name=f"acc{q}", tag=f"acc{q}")
            for q in range(2)
        ]

        def acc_ap(B, w=128):
            return accs[B // 4][:, (B % 4) * 128 : (B % 4) * 128 + w]

        o2 = ps_misc.tile([128, OFF2[NB - 1] + W2[NB - 1]], F32, name="o2", tag="o2")

        def emit_preloads():
            # rank-1 preloads, accumulated well after row 0 (row 0 carries the
            # start=True bank zeroing) so the strip DMA is off the critical
            # path: sbq, then -qb (host-masked), then +q_j for ACT rows
            for q in range(2):
                nc.tensor.matmul(
                    accs[q][:, :],
                    lhsT=srow(S_ONES, 128),
                    rhs=srow(S_SBQ + q * 512, 512),
                    start=False, stop=False, skip_group_check=True,
                )
                nc.tensor.matmul(
                    accs[q][:, :],
                    lhsT=srow(S_QB, 128),
                    rhs=srow(S_ONES, 512),
                    start=False, stop=False, skip_group_check=True,
                )
                nc.tensor.matmul(
                    accs[q][:, :],
                    lhsT=srow(S_MASK, 128),
                    rhs=srow(S_Q + q * 512, 512),
                    start=False, stop=False, skip_group_check=True,
                )
            # out2 psum bank preload of the -u_loc free-axis term
            for B in range(1, NB):
                nc.tensor.matmul(
                    o2[:, OFF2[B] : OFF2[B] + W2[B]],
                    lhsT=srow(S_ONES, 128),
                    rhs=srow(S_UL, W2[B]),
                    start=(B == 1), stop=False, skip_group_check=True,
                )

        out_dram = nc.dram_tensor("scores", [r, n], F32, kind="ExternalOutput")
        out2_dram = nc.dram_tensor("scores_t", [n, r], F32, kind="ExternalOutput")
        out_s = const.tile([128, n], F32)
        out2_s = const.tile([128, OFF2[NB - 1] + W2[NB - 1]], F32)

        # ---------------- main loop ----------------
        done = [0] * NB
        next_drain = 0
        for k in ORDER:
            A = k // G
            J0 = A * 128
            w = n - J0
            absd = [
                absd_pool.tile([128, n], BF16, name=f"absd{h}", tag=f"absd{h}")
                for h in range(H)
            ]
            for h in range(H):
                if ACT_ROW[k]:
                    nc.scalar.activation(
                        absd[h][:, 0:w], et(h)[:, J0:n], Act.Abs,
                        bias=ebt[:, h * 128 + k : h * 128 + k + 1], scale=-1.0,
                    )
                elif k == 0:
                    # row 0 gates the pipeline: produce it in j-halves so the
                    # first matmuls start right after the first ett DMA piece
                    for jh in range(2):
                        nc.vector.tensor_scalar(
                            out=absd[h][:, jh * 512 : (jh + 1) * 512],
                            in0=et(h)[:, jh * 512 : (jh + 1) * 512],
                            scalar1=ebt[:, h * 128 + k : h * 128 + k + 1],
                            scalar2=2.0, op0=Alu.max, op1=Alu.mult,
                        )
                else:
                    nc.vector.tensor_scalar(
                        out=absd[h][:, 0:w], in0=et(h)[:, J0:n],
                        scalar1=ebt[:, h * 128 + k : h * 128 + k + 1],
                        scalar2=2.0, op0=Alu.max, op1=Alu.mult,
                    )
            done[A] += 1
            last_row = ORDER[-1]
            for h in range(H):
                lw = win[:, h * 256 + 128 - k : h * 256 + 256 - k]
                for B in range(A, NB):
                    # row 0's first write per acc bank carries start=True (bank
                    # zeroing via pending-zero); preloads accumulate later.
                    # acc0's last write is row 63 (B=3); acc1's is the final
                    # processed row (every row writes B=7).
                    nc.tensor.matmul(
                        acc_ap(B),
                        lhsT=lw,
                        rhs=absd[h][:, B * 128 - J0 : (B + 1) * 128 - J0],
                        start=(k == 0 and h == 0 and B % 4 == 0),
                        stop=(h == H - 1 and ((k == 63 and B == 3) or (k == last_row and B == 7))),
                        skip_group_check=True,
                    )
            if k == 13:
                emit_preloads()

            # drain block B once every row of groups <= B has been processed
            while next_drain < NB and all(done[a] == G for a in range(next_drain + 1)):
                B = next_drain
                next_drain += 1
                nc.vector.tensor_scalar(
                    out=out_s[:, B * 128 : (B + 1) * 128], in0=acc_ap(B),
                    scalar1=sa_col[:, :], scalar2=None, op0=Alu.add,
                )
                nc.sync.dma_start(
                    out=out_dram.ap()[:, B * 128 : (B + 1) * 128],
                    in_=out_s[:, B * 128 : (B + 1) * 128],
                )
                if B >= 1:
                    nc.tensor.matmul(
                        o2[:, OFF2[B] : OFF2[B] + W2[B]],
                        lhsT=out_s[0 : W2[B], B * 128 : (B + 1) * 128],
                        rhs=ident32[0 : W2[B], 0 : W2[B]],
                        is_transpose=True,
                        start=False, stop=(B == NB - 1),
                        skip_group_check=True,
                    )
                    nc.vector.tensor_scalar(
                        out=out2_s[:, OFF2[B] : OFF2[B] + W2[B]],
                        in0=o2[:, OFF2[B] : OFF2[B] + W2[B]],
                        scalar1=u_nat[:, B : B + 1], scalar2=None, op0=Alu.add,
                    )
                    nc.sync.dma_start(
                        out=out2_dram.ap()[B * 128 : (B + 1) * 128, 0 : W2[B]],
                        in_=out2_s[:, OFF2[B] : OFF2[B] + W2[B]],
                    )

    nc.finalize()
    return nc


_CACHE: dict = {}


def _get_program() -> bass.Bass:
    if "nc" not in _CACHE:
        _CACHE["nc"] = build_program()
    return _CACHE["nc"]


def host_prep(emb: np.ndarray, W: np.ndarray, b: np.ndarray):
    """All O(N D) derived constants, per core. Returns (shared, per_core)."""
    import ml_dtypes

    BF = ml_dtypes.bfloat16
    n, d = emb.shape
    w = W[:, 0].astype(np.float32)
    wa, wb, wc = w[:d], w[d : 2 * d], w[2 * d :]
    awc = np.abs(wc)
    sgn = np.sign(wc).astype(np.float32)

    embbf = emb.astype(BF).astype(np.float32)          # the bf16 grid
    # ett[dp, h*n + j] = |wc|_{h*128+dp} * E~[j, h*128+dp]
    ettf = (awc[:, None] * embbf.T)                    # [256, n] f32
    ett = np.concatenate([ettf[:128, :], ettf[128:, :]], axis=1).astype(BF)
    ettf_q = ett.astype(np.float32)                    # quantized grid for q

    # q_j on the exact bf16 grid the PE consumes
    q = sgn[:128] @ ettf_q[:, :n] + sgn[128:] @ ettf_q[:, n:]
    sa = emb.astype(np.float32) @ wa
    sb = emb.astype(np.float32) @ wb
    u = sa - sb
    sbq = sb - q + b[0]

    win = np.zeros((128, H * 256), dtype=np.float32)
    for h in range(H):
        win[:, h * 256 + 128] = sgn[h * 128 : (h + 1) * 128]

    shared = {
        "ett": ett,
        "wint": win.astype(BF),
        "unat": np.ascontiguousarray(u.reshape(NB, 128).T.astype(np.float32)),
    }

    mask_act = np.array([1.0 if ACT_ROW[k] else 0.0 for k in range(R)], np.float32)
    per_core = []
    for c in range(N_CORES):
        gi = c + N_CORES * np.arange(R)
        ebloc = ettf_q.reshape(128, H, n)[:, :, :]      # [128, H, n]
        # ebt[dp, h*128 + k] = |wc| * E~[gi[k], h*128+dp]  (f32, bf16 grid)
        ebt = np.empty((128, H * 128), dtype=np.float32)
        for h in range(H):
            ebt[:, h * 128 : (h + 1) * 128] = ebloc[:, h, gi]
        qb = sgn[:128] @ ebt[:, :128] + sgn[128:] @ ebt[:, 128:]
        strip = np.zeros((1, S_LEN), dtype=np.float32)
        strip[0, S_SBQ : S_SBQ + n] = sbq
        strip[0, S_QB : S_QB + 128] = -qb * (1.0 - mask_act)
        strip[0, S_UL : S_UL + 128] = -u[gi]
        strip[0, S_Q : S_Q + n] = q
        strip[0, S_MASK : S_MASK + 128] = mask_act
        strip[0, S_ONES : S_ONES + 512] = 1.0
        per_core.append(
            {
                "ebtt": ebt,
                "strip": strip.astype(BF),
                "sacol": sa[gi].reshape(128, 1).astype(np.float32),
            }
        )
    return shared, per_core


def kernel(**inputs: np.ndarray) -> np.ndarray:
    emb = np.ascontiguousarray(np.asarray(inputs["utterance_embeddings"], dtype=np.float32))
    W = np.ascontiguousarray(np.asarray(inputs["W"], dtype=np.float32))
    b = np.ascontiguousarray(np.asarray(inputs["b"], dtype=np.float32))
    n = emb.shape[0]

    shared, per_core = host_prep(emb, W, b)
    nc = _get_program()
    in_maps = [{**shared, **per_core[c]} for c in range(N_CORES)]
    res = run_bass_kernel_spmd(nc, in_maps, list(range(N_CORES)))

    out = np.empty((n, n), dtype=np.float32)
    karr = np.arange(R)
    for c in range(N_CORES):
        out1 = res.results[c]["scores"]      # [R, n]
        out2 = res.results[c]["scores_t"]    # [n, R]
        gi = c + N_CORES * karr
        for A in range(NB):
            rws = gi[G * A : G * (A + 1)]
            out[rws, A * 128 :] = out1[G * A : G * (A + 1), A * 128 :]
        for B in range(1, NB):
            out[B * 128 : (B + 1) * 128, gi[: G * B]] = out2[
                B * 128 : (B + 1) * 128, : G * B
            ]
    return out


if __name__ == "__main__":
    rng = np.random.default_rng(0)
    emb = rng.standard_normal((N, D), dtype=np.float32)
    W = (rng.standard_normal((3 * D, 1), dtype=np.float32) / np.sqrt(3 * D)).astype(np.float32)
    b = np.zeros((1,), dtype=np.float32)
    out = kernel(utterance_embeddings=emb, W=W, b=b)
    print(out.shape, out.dtype)


# revision 6
# speedup vs baseline: 1.0004x; 1.0004x over previous
"""Trainium2 Bass kernel for pairwise-scores CoreNet — v3.

scores[i, j] = (e_i @ wa) + (e_j @ wb) + sum_d wc_d * |e_id - e_jd| + b

Device computes only the O(N^2 D) pair term; every O(N D) derived constant
(scaled transposed table, rank-1 rows, sign windows) is prepared host-side
and DMA'd in.

Structure (per core, rows interleaved i mod 8 == c):
  * upper block-triangle only: local row k computes cols j >= (k//16)*128;
    lower blocks come from on-device PE transposes of the computed blocks
    with the rank-1 asymmetry (u_j - u_i, u = sa - sb) patched in.
  * absd tiles: DVE computes 2*max(e~_dj, e~_dk) (corrections folded in the
    PSUM preload); every 4th wide row is produced on the ACT engine as
    |e~_dj - e~_dk| directly (no corrections; preload masked per row).
  * PE reduces over d via sliding one-hot sign windows into PSUM, one
    useful output row per matmul.
"""

import sys

sys.path.insert(0, "/opt/trn_rl_repo")

from contextlib import ExitStack

import numpy as np

import concourse.bass as bass
import concourse.mybir as mybir
import concourse.tile as tile
from concourse import bacc
from concourse.bass_utils import run_bass_kernel_spmd

F32 = mybir.dt.float32
BF16 = mybir.dt.bfloat16
Alu = mybir.AluOpType
Act = mybir.ActivationFunctionType

N_CORES = 8
N, D, R = 1024, 256, 128
H = D // 128          # 2 d-tiles
NB = N // 128         # 8 column blocks
G = R // NB           # 16 rows per block-group

# ACT rows: every 4th row (narrow A=7 fillers excluded); aligned so each
# post-drain row is an ACT row (frees the DVE for the drain copies). k=0
# stays on DVE: it gates the pipeline start and ACT is 3x slower per tile.
ACT_ROW = [k % 4 == 0 and 0 < k < 112 for k in range(R)]

# processing order: groups A=0..6 sequential (so PSUM blocks complete in
# order), with the 16 narrowest rows (A=7) interleaved one per 4 rows into
# the A=0..3 sections, which have DVE slack to absorb them
ORDER = []
for t in range(16):
    ORDER.extend(range(4 * t, 4 * (t + 1)))
    ORDER.append(112 + t)
ORDER.extend(range(64, 112))
assert sorted(ORDER) == list(range(128))

# strip layout (bf16 row constants)
S_SBQ = 0          # [0:1024)      sbq_j = sb_j - q_j + b
S_QB = 1024        # [1024:1152)   -qb_loc
S_UL = 1152        # [1152:1280)   -u_loc
S_Q = 1280         # [1280:2304)   q_j
S_MASK = 2304      # [2304:2432)   mask_act (1.0 on ACT rows)
S_ONES = 2432      # [2432:2944)   ones (512 wide: used as an F=512 rhs)
S_LEN = 2944

OFF2 = [8 * B * (B - 1) for B in range(NB)]   # out2 col offset per block B
W2 = [G * B for B in range(NB)]               # out2 width per block B


def build_program() -> bass.Bass:
    n, d, r = N, D, R
    nc = bacc.Bacc("TRN2", target_bir_lowering=False, debug=False)

    ett_dram = nc.dram_tensor("ett", [128, H * n], BF16, kind="ExternalInput")
    ebt_dram = nc.dram_tensor("ebtt", [128, H * 128], F32, kind="ExternalInput")
    strip_dram = nc.dram_tensor("strip", [1, S_LEN], BF16, kind="ExternalInput")
    unat_dram = nc.dram_tensor("unat", [128, NB], F32, kind="ExternalInput")
    sacol_dram = nc.dram_tensor("sacol", [128, 1], F32, kind="ExternalInput")
    win_dram = nc.dram_tensor("wint", [128, H * 256], BF16, kind="ExternalInput")
    ident_dram = nc.inline_tensor(np.eye(128, dtype=np.float32), name="ident128")

    with tile.TileContext(nc) as tc, ExitStack() as ctx:
        const = ctx.enter_context(tc.tile_pool(name="const", bufs=1))
        absd_pool = ctx.enter_context(tc.tile_pool(name="absd", bufs=5))
        ps_acc = ctx.enter_context(tc.tile_pool(name="psacc", bufs=1, space="PSUM"))
        ps_misc = ctx.enter_context(tc.tile_pool(name="psmisc", bufs=1, space="PSUM"))

        # ---------------- DMA loads (spread across 3 issuing engines) ----------------
        # critical chain: win + ett piece 0 + ebt -> absd(k=0) -> first
        # matmuls. ACT issues no DMAs (its 1.3us act-table load would delay
        # them); the rest ride the Pool queue.
        win = const.tile([128, H * 256], BF16)
        nc.sync.dma_start(out=win[:, :], in_=win_dram.ap())
        e_t2 = const.tile([128, H * n], BF16)      # e_t2[:, h*n + j]
        for piece in range(2):
            nc.sync.dma_start(
                out=e_t2[:, piece * 512 : (piece + 1) * 512],
                in_=ett_dram.ap()[:, piece * 512 : (piece + 1) * 512],
            )

        ebt = const.tile([128, H * 128], F32)      # ebt[:, h*128 + k]
        nc.gpsimd.dma_start(out=ebt[:, :], in_=ebt_dram.ap())
        for piece in range(2, 4):
            nc.gpsimd.dma_start(
                out=e_t2[:, piece * 512 : (piece + 1) * 512],
                in_=ett_dram.ap()[:, piece * 512 : (piece + 1) * 512],
            )
        rows = const.tile([1, S_LEN], BF16)
        nc.gpsimd.dma_start(out=rows[0:1, :], in_=strip_dram.ap())
        ident32 = const.tile([128, 128], F32)
        nc.gpsimd.dma_start(out=ident32[:, :], in_=ident_dram.ap())
        u_nat = const.tile([128, NB], F32)
        nc.gpsimd.dma_start(out=u_nat[:, :], in_=unat_dram.ap())
        sa_col = const.tile([128, 1], F32)
        nc.gpsimd.dma_start(out=sa_col[:, :], in_=sacol_dram.ap())

        def et(h):
            return e_t2[:, h * n : (h + 1) * n]

        def srow(off, w):
            return rows[0:1, off : off + w]

        # ---------------- PSUM preload ----------------
        accs = [
            ps_acc.tile([128, 512], F32, name=f"acc{q}", tag=f"acc{q}")
            for q in range(2)
        ]

        def acc_ap(B, w=128):
            return accs[B // 4][:, (B % 4) * 128 : (B % 4) * 128 + w]

        o2 = ps_misc.tile([128, OFF2[NB - 1] + W2[NB - 1]], F32, name="o2", tag="o2")

        def emit_preloads():
            # rank-1 preloads, accumulated well after row 0 (row 0 carries the
            # start=True bank zeroing) so the strip DMA is off the critical
            # path: sbq, then -qb (host-masked), then +q_j for ACT rows
            for q in range(2):
                nc.tensor.matmul(
                    accs[q][:, :],
                    lhsT=srow(S_ONES, 128),
                    rhs=srow(S_SBQ + q * 512, 512),
                    start=False, stop=False, skip_group_check=True,
                )
                nc.tensor.matmul(
                    accs[q][:, :],
                    lhsT=srow(S_QB, 128),
                    rhs=srow(S_ONES, 512),
                    start=False, stop=False, skip_group_check=True,
                )
                nc.tensor.matmul(
                    accs[q][:, :],
                    lhsT=srow(S_MASK, 128),
                    rhs=srow(S_Q + q * 512, 512),
                    start=False, stop=False, skip_group_check=True,
                )
            # out2 psum bank preload of the -u_loc free-axis term
            for B in range(1, NB):
                nc.tensor.matmul(
                    o2[:, OFF2[B] : OFF2[B] + W2[B]],
                    lhsT=srow(S_ONES, 128),
                    rhs=srow(S_UL, W2[B]),
                    start=(B == 1), stop=False, skip_group_check=True,
                )

        out_dram = nc.dram_tensor("scores", [r, n], F32, kind="ExternalOutput")
        out2_dram = nc.dram_tensor("scores_t", [n, r], F32, kind="ExternalOutput")
        out_s = const.tile([128, n], F32)
        out2_s = const.tile([128, OFF2[NB - 1] + W2[NB - 1]], F32)

        # ---------------- main loop ----------------
        done = [0] * NB
        next_drain = 0
        pending_t = []

        def flush_transposes():
            while pending_t:
                B = pending_t.pop(0)
                nc.tensor.matmul(
                    o2[:, OFF2[B] : OFF2[B] + W2[B]],
                    lhsT=out_s[0 : W2[B], B * 128 : (B + 1) * 128],
                    rhs=ident32[0 : W2[B], 0 : W2[B]],
                    is_transpose=True,
                    start=False, stop=(B == NB - 1),
                    skip_group_check=True,
                )
                nc.vector.tensor_scalar(
                    out=out2_s[:, OFF2[B] : OFF2[B] + W2[B]],
                    in0=o2[:, OFF2[B] : OFF2[B] + W2[B]],
                    scalar1=u_nat[:, B : B + 1], scalar2=None, op0=Alu.add,
                )
                nc.sync.dma_start(
                    out=out2_dram.ap()[B * 128 : (B + 1) * 128, 0 : W2[B]],
                    in_=out2_s[:, OFF2[B] : OFF2[B] + W2[B]],
                )

        for k in ORDER:
            A = k // G
            J0 = A * 128
            w = n - J0
            absd = [
                absd_pool.tile([128, n], BF16, name=f"absd{h}", tag=f"absd{h}")
                for h in range(H)
            ]
            for h in range(H):
                if ACT_ROW[k]:
                    nc.scalar.activation(
                        absd[h][:, 0:w], et(h)[:, J0:n], Act.Abs,
                        bias=ebt[:, h * 128 + k : h * 128 + k + 1], scale=-1.0,
                    )
                elif k == 0:
                    # row 0 gates the pipeline: produce it in j-halves so the
                    # first matmuls start right after the first ett DMA piece
                    for jh in range(2):
                        nc.vector.tensor_scalar(
                            out=absd[h][:, jh * 512 : (jh + 1) * 512],
                            in0=et(h)[:, jh * 512 : (jh + 1) * 512],
                            scalar1=ebt[:, h * 128 + k : h * 128 + k + 1],
                            scalar2=2.0, op0=Alu.max, op1=Alu.mult,
                        )
                else:
                    nc.vector.tensor_scalar(
                        out=absd[h][:, 0:w], in0=et(h)[:, J0:n],
                        scalar1=ebt[:, h * 128 + k : h * 128 + k + 1],
                        scalar2=2.0, op0=Alu.max, op1=Alu.mult,
                    )
            done[A] += 1
            last_row = ORDER[-1]
            for h in range(H):
                lw = win[:, h * 256 + 128 - k : h * 256 + 256 - k]
                for B in range(A, NB):
                    # row 0's first write per acc bank carries start=True (bank
                    # zeroing via pending-zero); preloads accumulate later.
                    # acc0's last write is row 63 (B=3); acc1's is the final
                    # processed row (every row writes B=7).
                    nc.tensor.matmul(
                        acc_ap(B),
                        lhsT=lw,
                        rhs=absd[h][:, B * 128 - J0 : (B + 1) * 128 - J0],
                        start=(k == 0 and h == 0 and B % 4 == 0),
                        stop=(h == H - 1 and ((k == 63 and B == 3) or (k == last_row and B == 7))),
                        skip_group_check=True,
                    )
            if k == 13:
                emit_preloads()

            # drain block B once every row of groups <= B has been processed.
            # The transpose chain is deferred one row (pending list) so the
            # next row's matmuls aren't blocked behind the DVE drain copy.
            flush_transposes()
            while next_drain < NB and all(done[a] == G for a in range(next_drain + 1)):
                B = next_drain
                next_drain += 1
                nc.vector.tensor_scalar(
                    out=out_s[:, B * 128 : (B + 1) * 128], in0=acc_ap(B),
                    scalar1=sa_col[:, :], scalar2=None, op0=Alu.add,
                )
                nc.sync.dma_start(
                    out=out_dram.ap()[:, B * 128 : (B + 1) * 128],
                    in_=out_s[:, B * 128 : (B + 1) * 128],
                )
                if B >= 1:
                    pending_t.append(B)
        flush_transposes()

    nc.finalize()
    return nc


_CACHE: dict = {}


def _get_program() -> bass.Bass:
    if "nc" not in _CACHE:
        _CACHE["nc"] = build_program()
    return _CACHE["nc"]


def host_prep(emb: np.ndarray, W: np.ndarray, b: np.ndarray):
    """All O(N D) derived constants, per core. Returns (shared, per_core)."""
    import ml_dtypes

    BF = ml_dtypes.bfloat16
    n, d = emb.shape
    w = W[:, 0].astype(np.float32)
    wa, wb, wc = w[:d], w[d : 2 * d], w[2 * d :]
    awc = np.abs(wc)
    sgn = np.sign(wc).astype(np.float32)

    embbf = emb.astype(BF).astype(np.float32)          # the bf16 grid
    # ett[dp, h*n + j] = |wc|_{h*128+dp} * E~[j, h*128+dp]
    ettf = (awc[:, None] * embbf.T)                    # [256, n] f32
    ett = np.concatenate([ettf[:128, :], ettf[128:, :]], axis=1).astype(BF)
    ettf_q = ett.astype(np.float32)                    # quantized grid for q

    # q_j on the exact bf16 grid the PE consumes
    q = sgn[:128] @ ettf_q[:, :n] + sgn[128:] @ ettf_q[:, n:]
    sa = emb.astype(np.float32) @ wa
    sb = emb.astype(np.float32) @ wb
    u = sa - sb
    sbq = sb - q + b[0]

    win = np.zeros((128, H * 256), dtype=np.float32)
    for h in range(H):
        win[:, h * 256 + 128] = sgn[h * 128 : (h + 1) * 128]

    shared = {
        "ett": ett,
        "wint": win.astype(BF),
        "unat": np.ascontiguousarray(u.reshape(NB, 128).T.astype(np.float32)),
    }

    mask_act = np.array([1.0 if ACT_ROW[k] else 0.0 for k in range(R)], np.float32)
    per_core = []
    for c in range(N_CORES):
        gi = c + N_CORES * np.arange(R)
        ebloc = ettf_q.reshape(128, H, n)[:, :, :]      # [128, H, n]
        # ebt[dp, h*128 + k] = |wc| * E~[gi[k], h*128+dp]  (f32, bf16 grid)
        ebt = np.empty((128, H * 128), dtype=np.float32)
        for h in range(H):
            ebt[:, h * 128 : (h + 1) * 128] = ebloc[:, h, gi]
        qb = sgn[:128] @ ebt[:, :128] + sgn[128:] @ ebt[:, 128:]
        strip = np.zeros((1, S_LEN), dtype=np.float32)
        strip[0, S_SBQ : S_SBQ + n] = sbq
        strip[0, S_QB : S_QB + 128] = -qb * (1.0 - mask_act)
        strip[0, S_UL : S_UL + 128] = -u[gi]
        strip[0, S_Q : S_Q + n] = q
        strip[0, S_MASK : S_MASK + 128] = mask_act
        strip[0, S_ONES : S_ONES + 512] = 1.0
        per_core.append(
            {
                "ebtt": ebt,
                "strip": strip.astype(BF),
                "sacol": sa[gi].reshape(128, 1).astype(np.float32),
            }
        )
    return shared, per_core


def kernel(**inputs: np.ndarray) -> np.ndarray:
    emb = np.ascontiguousarray(np.asarray(inputs["utterance_embeddings"], dtype=np.float32))
    W = np.ascontiguousarray(np.asarray(inputs["W"], dtype=np.float32))
    b = np.ascontiguousarray(np.asarray(inputs["b"], dtype=np.float32))
    n = emb.shape[0]

    shared, per_core = host_prep(emb, W, b)
    nc = _get_program()
    in_maps = [{**shared, **per_core[c]} for c in range(N_CORES)]
    res = run_bass_kernel_spmd(nc, in_maps, list(range(N_CORES)))

    out = np.empty((n, n), dtype=np.float32)
    karr = np.arange(R)
    for c in range(N_CORES):
        out1 = res.results[c]["scores"]      # [R, n]
        out2 = res.results[c]["scores_t"]    # [n, R]
        gi = c + N_CORES * karr
        for A in range(NB):
            rws = gi[G * A : G * (A + 1)]
            out[rws, A * 128 :] = out1[G * A : G * (A + 1), A * 128 :]
        for B in range(1, NB):
            out[B * 128 : (B + 1) * 128, gi[: G * B]] = out2[
                B * 128 : (B + 1) * 128, : G * B
            ]
    return out


if __name__ == "__main__":
    rng = np.random.default_rng(0)
    emb = rng.standard_normal((N, D), dtype=np.float32)
    W = (rng.standard_normal((3 * D, 1), dtype=np.float32) / np.sqrt(3 * D)).astype(np.float32)
    b = np.zeros((1,), dtype=np.float32)
    out = kernel(utterance_embeddings=emb, W=W, b=b)
    print(out.shape, out.dtype)
